# revision 5
# baseline (speedup 1.0000x reference)
"""Trainium2 Bass kernel for a CapsNet routing sublayer.

Reference computation (per problem statement):
    u_hat[b,c,u,s] = sum_i W[0,c,u,s,i] * x[b,i,c]
    3 rounds of dynamic routing over batch-summed s_j, returning v_j[c,u,s].

Key algebraic facts exploited here:
  * c_ij / v_j are batch-independent, and the agreement reduction
    factorizes through sum_b, so u_hat never needs materializing:
        u_sum[c,u,s] = sum_i W[c,u,s,i] * xs[c,i],  xs[c,i] = sum_b x[b,i,c]
  * Routing is independent per capsule channel c -> shard C=1152 across
    8 cores with zero collectives.
  * softmax / squash / agreement combine into a division-light form:
        e = exp(b); Z = sum_u e; st = e*u_sum
        v = st * sqrt(|st|^2) / (Z^2 + |st|^2)
        b += <u_sum/B, st> * sqrt(|st|^2) / (Z^2 + |st|^2)

Per-core layout: the 144 local channels sit two-per-partition on 72 SBUF
partitions; (u,s)=160 values per channel live in the free dimension.
The batch reduction runs on the tensor engine (ones-matmul, PSUM
accumulate over the two 128-row batch tiles).
"""

import numpy as np

import concourse.bass as bass
import concourse.bacc as bacc
import concourse.mybir as mybir
import concourse.tile as tile
from concourse.bass_utils import run_bass_kernel_spmd

N_CORES = 8
B, I, C, U, S = 256, 8, 1152, 10, 16
CL = C // N_CORES          # 144 channels per core
CP = CL // 2               # 72 partitions, 2 channels each
F = U * S                  # 160
NUM_ROUTING = 3

_cache: dict = {}


def _build_nc():
    if "nc" in _cache:
        return _cache["nc"]

    dt = mybir.dt.float32
    AF = mybir.ActivationFunctionType
    OP = mybir.AluOpType
    AX = mybir.AxisListType

    nc = bacc.Bacc("TRN2")
    x_d = nc.dram_tensor("x_s", [B, I, CL], dt, kind="ExternalInput")
    w_d = nc.dram_tensor("w_s", [CL, U, S, I], dt, kind="ExternalInput")
    v_d = nc.dram_tensor("v", [CL, U, S], dt, kind="ExternalOutput")

    with tile.TileContext(nc) as tc:
        with (
            tc.tile_pool(name="main", bufs=1) as pool,
            tc.tile_pool(name="psum", bufs=1, space="PSUM") as psum,
        ):
            # ---- phase 1: xs[c,i] = sum_b x[b,i,c] via ones-matmul ----
            ones = pool.tile([128, 1], dt)
            nc.vector.memset(ones[:], 1.0)

            xt0 = pool.tile([128, I * CL], dt)
            xt1 = pool.tile([128, I * CL], dt)
            nc.sync.dma_start(
                xt0[:], x_d[0:128].rearrange("b i c -> b (i c)")
            )
            nc.sync.dma_start(
                xt1[:], x_d[128:256].rearrange("b i c -> b (i c)")
            )

            # psum_xs free layout: c-major, i-minor (via reordered rhs AP)
            psum_xs = psum.tile([1, I * CL], dt)
            # free slices (in c units) each <=512 elements and bank-aligned
            c_slices = [(0, 64), (64, 64), (128, 16)]
            for j, xt in enumerate((xt0, xt1)):
                xv = xt[:].rearrange("p (i c) -> p c i", i=I)  # [128, 144, 8]
                for lo, sz in c_slices:
                    nc.tensor.matmul(
                        psum_xs[:1, lo * I:(lo + sz) * I],
                        ones[:],
                        xv[:, lo:lo + sz, :],
                        start=(j == 0),
                        stop=(j == 1),
                    )

            # bounce PSUM->SBUF (DMA cannot read PSUM), then scatter
            # [1, (c,i)] -> [72, (2,8)] (c-pair per partition)
            xs_row = pool.tile([1, I * CL], dt)
            nc.scalar.copy(xs_row[:], psum_xs[:1, :])
            xs = pool.tile([CP, 2 * I], dt)
            nc.sync.dma_start(
                xs[:], xs_row[:].rearrange("a (p f) -> a p f", f=2 * I)
            )

            # ---- phase 2: u_sum[c,u,s] = sum_i w[c,u,s,i]*xs[c,i] ----
            wt = pool.tile([CP, 2 * F * I], dt)
            nc.sync.dma_start(
                wt[:], w_d[:].rearrange("(p a) u s i -> p (a u s i)", a=2)
            )
            def bcast_dim(ap_obj, axis, count):
                dims = [list(d) for d in ap_obj.ap]
                dims.insert(axis, [0, count])
                return bass.AP(ap_obj.tensor, ap_obj.offset, dims)

            w4 = wt[:].rearrange("p (a f i) -> p a f i", a=2, i=I)  # [72,2,160,8]
            xs3 = xs[:].rearrange("p (a i) -> p a i", a=2)          # [72,2,8]
            xs_bc = bcast_dim(xs3, 2, F)                            # [72,2,160,8]

            prod = pool.tile([CP, 2 * F * I], dt)
            prod4 = prod[:].rearrange("p (a f i) -> p a f i", a=2, i=I)
            nc.vector.tensor_tensor(prod4, w4, xs_bc, op=OP.mult)

            us = pool.tile([CP, 2 * F], dt)
            nc.vector.tensor_reduce(us[:], prod4, axis=AX.X, op=OP.add)

            us4 = us[:].rearrange("p (a u s) -> p a u s", a=2, s=S)   # [72,2,10,16]
            us3 = us[:].rearrange("p (a u s) -> p (a u) s", a=2, s=S)  # [72,20,16]

            # ---- phase 3: routing ----
            st = pool.tile([CP, 2 * F], dt)     # e * u_sum
            q = pool.tile([CP, 2 * F], dt)      # st*st
            g = pool.tile([CP, 2 * F], dt)      # (u_sum/B)*st
            vt = pool.tile([CP, 2 * F], dt)     # final output
            m2 = pool.tile([CP, 2 * U], dt)
            h = pool.tile([CP, 2 * U], dt)
            bb = pool.tile([CP, 2 * U], dt)     # routing logits
            e = pool.tile([CP, 2 * U], dt)
            D = pool.tile([CP, 2 * U], dt)
            R = pool.tile([CP, 2 * U], dt)
            mag = pool.tile([CP, 2 * U], dt)
            t = pool.tile([CP, 2 * U], dt)
            z = pool.tile([CP, 2], dt)
            zsq = pool.tile([CP, 2], dt)

            def views(tl):
                return (
                    tl[:].rearrange("p (a u s) -> p a u s", a=2, s=S),
                    tl[:].rearrange("p (a u s) -> p (a u) s", a=2, s=S),
                )

            st4, st3 = views(st)
            q4, q3 = views(q)
            g4, g3 = views(g)
            v4, _ = views(vt)
            m2_3 = m2[:].rearrange("p (a u) -> p a u", a=2)
            e3 = e[:].rearrange("p (a u) -> p a u", a=2)
            e_bc = bcast_dim(e3, 3, S)                   # [72,2,10,16]
            zsq_bc = bcast_dim(zsq[:], 2, U)             # [72,2,10]
            F_bc = bcast_dim(t[:].rearrange("p (a u) -> p a u", a=2), 3, S)

            # iteration 1: b=0 -> e=1, Z=U, st=u_sum
            nc.vector.tensor_tensor(q[:], us[:], us[:], op=OP.mult)
            nc.vector.tensor_reduce(m2[:], q3, axis=AX.X, op=OP.add)
            nc.vector.tensor_scalar_add(D[:], m2[:], float(U * U))
            nc.vector.reciprocal(R[:], D[:])
            nc.scalar.sqrt(mag[:], m2[:])
            nc.vector.tensor_tensor(t[:], mag[:], R[:], op=OP.mult)
            # b1 = (m2/B) * t   (h1 = <u_sum/B, u_sum> = m2/B)
            nc.vector.scalar_tensor_tensor(
                bb[:], m2[:], 1.0 / B, t[:], op0=OP.mult, op1=OP.mult
            )

            for it in range(1, NUM_ROUTING):
                last = it == NUM_ROUTING - 1
                nc.scalar.activation(e[:], bb[:], AF.Exp)
                nc.vector.tensor_reduce(z[:], e3, axis=AX.X, op=OP.add)
                nc.vector.tensor_tensor(zsq[:], z[:], z[:], op=OP.mult)
                nc.vector.tensor_tensor(st4, us4, e_bc, op=OP.mult)
                nc.vector.tensor_tensor(q[:], st[:], st[:], op=OP.mult)
                nc.vector.tensor_reduce(m2[:], q3, axis=AX.X, op=OP.add)
                nc.vector.tensor_tensor(D[:], m2_3, zsq_bc, op=OP.add)
                nc.vector.reciprocal(R[:], D[:])
                nc.scalar.sqrt(mag[:], m2[:])
                # t = mag * R  (overwrites t; also serves as F on last iter)
                nc.vector.tensor_tensor(t[:], mag[:], R[:], op=OP.mult)
                if not last:
                    # g = (u_sum/B) * st ; h = sum_s g
                    nc.vector.scalar_tensor_tensor(
                        g[:], us[:], 1.0 / B, st[:], op0=OP.mult, op1=OP.mult
                    )
                    nc.vector.tensor_reduce(h[:], g3, axis=AX.X, op=OP.add)
                    # b += h * t
                    nc.vector.tensor_tensor(h[:], h[:], t[:], op=OP.mult)
                    nc.vector.tensor_tensor(bb[:], bb[:], h[:], op=OP.add)
                else:
                    nc.vector.tensor_tensor(v4, st4, F_bc, op=OP.mult)

            nc.sync.dma_start(
                v_d[:].rearrange("(p a) u s -> p (a u s)", a=2), vt[:]
            )

    nc.compile()
    _cache["nc"] = nc
    return nc


def kernel(x: np.ndarray, weight: np.ndarray) -> np.ndarray:
    nc = _build_nc()
    in_maps = []
    for k in range(N_CORES):
        c0 = k * CL
        in_maps.append({
            "x_s": np.ascontiguousarray(x[:, :, c0:c0 + CL], dtype=np.float32),
            "w_s": np.ascontiguousarray(weight[0, c0:c0 + CL], dtype=np.float32),
        })
    res = run_bass_kernel_spmd(nc, in_maps, core_ids=list(range(N_CORES)))
    _cache["last_result"] = res
    return np.concatenate([r["v"] for r in res.results], axis=0)
